# revision 38
# baseline (speedup 1.0000x reference)
"""Causal self-attention (B=2, T=2048, C=1024, H=16, D=64) on 8 trn2 cores.

Sharding v3: DP on batch x TP on heads. Core c handles batch c//4 and the
4 heads [4*(c%4), 4*(c%4)+4), as two head-pairs hp0/hp1. Each core reads
only its batch's x (4.2MB bf16, half of v2), computes QKV for its 768
features, runs causal attention per head-pair, and projects through its
256 rows of proj_w into a [T, C] partial summed on host (+ proj bias).

Per-core layout:
  - QKV: weight-stationary chains - one w chunk [128,128] streams 1-2
    token-group rhs of N=512 into separate PSUM banks; bias applied by
    DVE (tensor_scalar_add) or ACT (upfront, while exp is idle).
  - S^T per (hp, q-subtile, k-chunk): 2 heads row-tiled (K=64 halves) into
    st [128, 2, 512] fp32 (2 banks); ONE exp per chunk on ACT covers both
    heads; causal triangle via DVE mul with doubled tri mask.
  - O^T: 2 heads COL-tiled (M=64 at tile_position (0,0)/(0,64)) into one
    ot [128, 512] bank, accumulating over k-chunks. V comes straight from
    DMA XBAR transpose quarters (no ones column, no DVE fixup).
  - Softmax denominator: DVE accumulates masked exp chunks into dacc
    [128, 2, 512] f32; GpSimd partition_all_reduce gives per-q sums
    broadcast to all partitions; DVE reciprocal + mul normalize ot
    (evacuated early to SBUF to free the PSUM bank).
  - Projection: per 128-token strip, K=256 over both hps' un tiles
    accumulated in PSUM, ct-pair weight-stationary; bf16 out DMA.
"""

from collections import deque

import numpy as np

import concourse.bass as bass
import concourse.tile as tile
from concourse import bacc, bass_isa, mybir
from concourse.bass_utils import run_bass_kernel_spmd

dt = mybir.dt
AF = mybir.ActivationFunctionType

B, T, C, H, D = 2, 2048, 1024, 16, 64
NCORES = 8
HPC = 4                    # heads per core
QS = 512                   # q-subtile (columns of S^T/O^T psum tiles)
KC = 128                   # k chunk (partition dim of S^T)
SCALE = 1.0 / 8.0          # 1/sqrt(D)

_CACHE = {}


def _emit(tc):
    from contextlib import ExitStack
    with ExitStack() as ctx:
        _emit_body(tc, ctx)


def _emit_body(tc, ctx):
    nc = tc.nc
    f32, bf16 = dt.float32, dt.bfloat16

    fp8 = dt.float8e4
    xT = nc.dram_tensor("xT", [C, T], fp8, kind="ExternalInput").ap()
    xTb = nc.dram_tensor("xTb", [C, T], bf16, kind="ExternalInput").ap()
    wqkv = nc.dram_tensor("wqkv", [C, 512], fp8, kind="ExternalInput").ap()
    wv = nc.dram_tensor("wv", [C, 256], bf16, kind="ExternalInput").ap()
    bqkv = nc.dram_tensor("bqkv", [128, 6], f32, kind="ExternalInput").ap()
    wp = nc.dram_tensor("wp", [128, 2, C], bf16, kind="ExternalInput").ap()
    tri2 = nc.dram_tensor("tri2", [128, 256], bf16, kind="ExternalInput").ap()
    outp = nc.dram_tensor("outp", [T, C], bf16, kind="ExternalOutput").ap()

    consts = ctx.enter_context(tc.tile_pool(name="consts", bufs=1))
    xpool = ctx.enter_context(tc.tile_pool(name="xpool", bufs=1))
    qkvpool = ctx.enter_context(tc.tile_pool(name="qkvpool", bufs=6))
    vtpool = ctx.enter_context(tc.tile_pool(name="vtpool", bufs=4))
    ptpool = ctx.enter_context(tc.tile_pool(name="ptpool", bufs=3))
    daccp = ctx.enter_context(tc.tile_pool(name="daccp", bufs=2))
    rcp = ctx.enter_context(tc.tile_pool(name="rcp", bufs=2))
    rbp = ctx.enter_context(tc.tile_pool(name="rbp", bufs=2))
    otbp = ctx.enter_context(tc.tile_pool(name="otbp", bufs=2))
    unp = ctx.enter_context(tc.tile_pool(name="unp", bufs=6))
    outsb = ctx.enter_context(tc.tile_pool(name="outsb", bufs=4))
    stp = ctx.enter_context(tc.tile_pool(name="stp", bufs=2, space="PSUM"))
    otp = ctx.enter_context(tc.tile_pool(name="otp", bufs=1, space="PSUM"))
    miscp = ctx.enter_context(tc.tile_pool(name="miscp", bufs=2, space="PSUM"))

    w_sb = consts.tile([128, 8, 512], fp8, tag="w")
    wv_sb = consts.tile([128, 8, 256], bf16, tag="wvt")
    ones_sb = consts.tile([128, 1], bf16, tag="ones")
    nc.vector.memset(ones_sb[:, :], 1.0)
    wsrc = wqkv.rearrange("(k p) f -> p k f", p=128)
    wvsrc = wv.rearrange("(k p) f -> p k f", p=128)
    b_sb = consts.tile([128, 6], f32, tag="b")
    wp_sb = consts.tile([128, 2, C], bf16, tag="wp")
    tri_sb = consts.tile([128, 256], bf16, tag="tri")

    x_sb = xpool.tile([128, 8, T], fp8, tag="xp")
    xv_sb = xpool.tile([128, 8, T], bf16, tag="xv")
    xsrc = xT.rearrange("(j p) t -> p j t", p=128)
    xvsrc = xTb.rearrange("(j p) t -> p j t", p=128)
    # batched enqueues spread across the three DMA-capable engines so the
    # first-needed transfers are all in flight within ~2us.
    nc.sync.dma_start(out=w_sb[:, :, :], in_=wsrc[:, :, :])
    nc.gpsimd.dma_start(out=x_sb[:, :, 0:512], in_=xsrc[:, :, 0:512])
    nc.scalar.dma_start(out=b_sb, in_=bqkv)
    nc.scalar.dma_start(out=x_sb[:, :, 512:1024], in_=xsrc[:, :, 512:1024])
    nc.gpsimd.dma_start(out=xv_sb[:, :, 0:512], in_=xvsrc[:, :, 0:512])
    nc.sync.dma_start(out=wv_sb[:, :, :], in_=wvsrc[:, :, :])
    nc.gpsimd.dma_start(out=tri_sb, in_=tri2)
    nc.scalar.dma_start(out=xv_sb[:, :, 512:1024], in_=xvsrc[:, :, 512:1024])
    nc.sync.dma_start(out=x_sb[:, :, 1024:2048], in_=xsrc[:, :, 1024:2048])
    nc.gpsimd.dma_start(out=xv_sb[:, :, 1024:2048],
                        in_=xvsrc[:, :, 1024:2048])
    nc.sync.dma_start(out=wp_sb, in_=wp)

    filler = deque()

    def pop_filler(n=1):
        for _ in range(n):
            if filler:
                filler.popleft()[1]()

    def drain_while(pred):
        """Pop everything up to and including the last item whose key
        matches `pred`."""
        if not any(pred(k) for k, _ in filler):
            return
        while filler:
            k, th = filler.popleft()
            th()
            if pred(k) and not any(pred(k2) for k2, _ in filler):
                break

    # qkv_t[(hp, m)] : [128, T] bf16; m: 0=q, 1=k, 2=v. features = 2 heads
    # of head-pair hp (rows 64h within the tile).
    qkv_t = {}
    for hp in range(2):
        for m in range(3):
            qkv_t[(hp, m)] = qkvpool.tile(
                [128, T], bf16, tag="qkv", name=f"qkv{hp}_{m}")

    def make_chain(hp, m, tgs, bias_on_act=False):
        """Weight-stationary QKV chain: one w load streams len(tgs)
        token-group rhs. q/k (m<2) run fp8 DoubleRow with K=256 per matmul
        (weights pre-scaled by 32 on host for fp8 subnormal avoidance; the
        bias step rescales). v runs bf16 for output-path precision."""
        fg = hp * 3 + m
        dst = qkv_t[(hp, m)]
        is8 = m < 2
        scale = 1.0 / 32.0 if is8 else 1.0
        state = {}

        def mk_mm(kk):
            def th():
                if kk == 0:
                    state["pg"] = [
                        miscp.tile([128, 512], f32, tag="mm", name="pg")
                        for _ in tgs]
                for i, tg in enumerate(tgs):
                    tsl = slice(512 * tg, 512 * tg + 512)
                    if is8:
                        f0 = 128 * (2 * hp + m)
                        nc.tensor.matmul(
                            state["pg"][i][:, :],
                            w_sb[:, 2 * kk:2 * kk + 2, f0:f0 + 128],
                            x_sb[:, 2 * kk:2 * kk + 2, tsl],
                            start=(kk == 0), stop=(kk == 3),
                            perf_mode=mybir.MatmulPerfMode.DoubleRow,
                        )
                    else:
                        f0 = 128 * hp
                        nc.tensor.matmul(
                            state["pg"][i][:, :],
                            wv_sb[:, kk, f0:f0 + 128],
                            xv_sb[:, kk, tsl],
                            start=(kk == 0), stop=(kk == 7),
                        )
            return th

        def th_bias():
            for i, tg in enumerate(tgs):
                o = dst[:, 512 * tg:512 * tg + 512]
                if bias_on_act:
                    nc.scalar.activation(
                        o, state["pg"][i][:, :], AF.Identity,
                        bias=b_sb[:, fg:fg + 1], scale=scale)
                else:
                    nc.vector.tensor_scalar(
                        o, state["pg"][i][:, :], scale,
                        b_sb[:, fg:fg + 1],
                        mybir.AluOpType.mult, mybir.AluOpType.add)
        return [mk_mm(kk) for kk in range(4 if is8 else 8)] + [th_bias]

    # vts[(hp, h)] : [128, 16, 64] token-major V via DMA XBAR transpose.
    vts = {}
    for hp in range(2):
        for h in range(2):
            vts[(hp, h)] = vtpool.tile(
                [128, 16, 64], bf16, tag="vt", name=f"vt{hp}_{h}")

    # V-transpose parts: token ranges sized so each part's source tokens
    # are covered by the QKV chains emitted before it.
    VT_PARTS = {0: (0, 512), 1: (512, 1024), 2: (1024, 2048)}

    def make_vt(hp, h, part):
        def th():
            t0, t1 = VT_PARTS[part]
            nc.sync.dma_start_transpose(
                vts[(hp, h)][:, t0 // 128:t1 // 128, :],
                qkv_t[(hp, 2)][64 * h:64 * h + 64, t0:t1])
        return th

    un_t = {}

    def emit_attention(hp, s, fin_prev=None):
        qd, kd = qkv_t[(hp, 0)], qkv_t[(hp, 1)]
        q0 = s * QS
        nkc = 4 * (s + 1)
        # col-tiled O pair: h0 accumulates in bank 0 (partitions 0-63),
        # h1 in bank 1 (partitions 64-127) - separate banks so the two
        # concurrent accumulation groups have disjoint zero regions.
        ot = otp.tile([128, 2, QS], f32, tag="ot", name="ot")
        dacc = daccp.tile([128, 2, QS], bf16, tag="dacc", name="dacc")

        def emit_o(kc):
            ls = max(0, kc * KC - q0)
            last = kc == nkc - 1
            pt = pts[kc]
            for h in range(2):
                nc.tensor.matmul(
                    ot[64 * h:64 * h + 64, h, ls:QS],
                    vts[(hp, h)][:, kc, :], pt[:, h, ls:QS],
                    start=(kc == 0), stop=last, tile_position=(0, 64 * h))

        pts = {}
        for kc in range(nkc):
            k0 = kc * KC
            ls = max(0, k0 - q0)
            st = stp.tile([128, 2, QS], f32, tag="st")
            for h in range(2):
                nc.tensor.matmul(
                    st[:, h, ls:QS],
                    kd[64 * h:64 * h + 64, k0:k0 + KC],
                    qd[64 * h:64 * h + 64, q0 + ls:q0 + QS],
                    start=True, stop=True, tile_position=(64 * h, 0))
            pt = ptpool.tile([128, 2, QS], bf16, tag="pt")
            pts[kc] = pt
            nc.scalar.activation(
                pt[:, :, ls:QS], st[:, :, ls:QS], AF.Exp, scale=SCALE)
            if kc >= 4 * s:  # diagonal chunk: zero invalid triangle, 2 heads
                nc.vector.tensor_mul(
                    pt[:, :, ls:ls + 128], pt[:, :, ls:ls + 128],
                    tri_sb[:, :])
            # denominator accumulation (masked exp) on DVE
            if kc == 0:
                nc.vector.tensor_copy(dacc[:, :, :], pt[:, :, :])
            else:
                nc.vector.tensor_add(
                    dacc[:, :, ls:QS], dacc[:, :, ls:QS], pt[:, :, ls:QS])
            # previous iteration's denominator/normalize is emitted here,
            # behind two S-pairs, so its dacc-dependent matmuls don't
            # head-of-line-block this iteration's S chunks on the PE queue
            # (and before emit_o(0) below reuses the single ot buffer).
            # Filler pops wait for it too: proj thunks need the un tiles.
            pending = fin_prev is not None
            if kc == 1 and pending:
                fin_prev()
                fin_prev = None
            # O lags one chunk so exp(kc) overlaps PE work
            if kc > 0:
                emit_o(kc - 1)
                pts.pop(kc - 1)
            if kc == 1 and pending:
                pop_filler(4)
            elif kc > 0 or not pending:
                pop_filler(2)
        emit_o(nkc - 1)
        pop_filler()

        def finalize():
            # denominator: ones.T @ dacc per head into small scratch PSUM
            # tiles (partition 0, plain tile position).
            dens = [miscp.tile([1, QS], f32, tag="mm", name=f"den{h}")
                    for h in range(2)]
            for h in range(2):
                nc.tensor.matmul(
                    dens[h][:, :], ones_sb[:, :], dacc[:, h, :],
                    start=True, stop=True)
            rc = rcp.tile([1, 2, QS], f32, tag="rc", name="rc")
            for h in range(2):
                nc.vector.reciprocal_approx_fast(rc[:, h, :], dens[h][:, :])
            # engines need matching base partitions across INPUTS: head0's
            # O^T (psum base 0) multiplies in place; head1's (base 64) goes
            # through a base-0 SBUF staging copy (in->out shift is fine).
            otb1 = otbp.tile([64, QS], f32, tag="otb1", name="otb1")
            nc.vector.tensor_copy(otb1[:, :], ot[64:128, 1, :])
            un = unp.tile([128, QS], bf16, tag="un", name=f"un{hp}{s}")
            for h, src in ((0, ot[0:64, 0, :]), (1, otb1[:, :])):
                rb = rbp.tile([64, QS], f32, tag=f"rb{h}", name=f"rb{h}")
                nc.gpsimd.partition_broadcast(rb[:, :], rc[:, h, :])
                nc.vector.tensor_mul(un[64 * h:64 * h + 64, :], src, rb[:, :])
            un_t[(hp, s)] = un
        return finalize

    def make_proj(s):
        """Projection thunks for one 128-token strip: K=256 over both hps.
        Evacuation casts go to ACT for the first/last subtiles (DVE relief
        where exp is idle), DVE otherwise."""
        thunks = []
        for ts in range(QS // 128):
            def th(ts=ts):
                a0 = s * QS + ts * 128
                un0, un1 = un_t[(0, s)], un_t[(1, s)]
                pp = [miscp.tile([128, 512], f32, tag="mm", name="pp")
                      for _ in range(2)]
                for hp, un in ((0, un0), (1, un1)):
                    for ct in range(2):
                        nc.tensor.matmul(
                            pp[ct][:, :],
                            un[:, ts * 128:(ts + 1) * 128],
                            wp_sb[:, hp, ct * 512:(ct + 1) * 512],
                            start=(hp == 0), stop=(hp == 1),
                        )
                for ct in range(2):
                    ob = outsb.tile([128, 512], bf16, tag="osb")
                    if s == 2:  # tail strips: ACT is idle by then
                        nc.scalar.copy(ob[:, :], pp[ct][:, :])
                    else:
                        nc.vector.tensor_copy(ob[:, :], pp[ct][:, :])
                    nc.sync.dma_start(
                        out=outp[a0:a0 + 128, ct * 512:(ct + 1) * 512],
                        in_=ob[:, :])
            thunks.append(th)
        return thunks

    # upfront: hp0 token-group-0 QKV (k, q, v) + V-transpose part 0,
    # emitted densely; these pipeline behind the input DMA stream.
    for m in (1, 0, 2):
        for th in make_chain(0, m, (0,), bias_on_act=True):
            th()
    make_vt(0, 0, 0)()
    make_vt(0, 1, 0)()

    # filler: remaining QKV/vt work, phased ahead of consumption. Chains
    # are keyed by the lowest token-group they cover; vt by part.
    def queue_chain(hp, m, tgs, key_tg):
        for th in make_chain(hp, m, tgs):
            filler.append((("qkv", hp, key_tg), th))

    for m in (1, 0, 2):
        queue_chain(1, m, (0, 1), 0)
    for h in range(2):
        filler.append((("vt", 1, 0), make_vt(1, h, 0)))
    for m in (1, 0, 2):
        queue_chain(0, m, (1,), 1)
    for h in range(2):
        filler.append((("vt", 0, 1), make_vt(0, h, 1)))
    for h in range(2):
        filler.append((("vt", 1, 1), make_vt(1, h, 1)))
    for m in (1, 0, 2):
        queue_chain(0, m, (2, 3), 2)
    for h in range(2):
        filler.append((("vt", 0, 2), make_vt(0, h, 2)))
    for m in (1, 0, 2):
        queue_chain(1, m, (2, 3), 2)
    for h in range(2):
        filler.append((("vt", 1, 2), make_vt(1, h, 2)))

    # iteration order keeps the filler-less closing iterations small:
    # s=3 runs mid-stream (its projection then feeds the s=2 tail).
    held = []
    fin = None
    for s in (0, 1, 3, 2):
        for hp in range(2):
            drain_while(lambda k, hp=hp, s=s: (
                k[0] == "qkv" and k[1] == hp and k[2] <= s))
            drain_while(lambda k, hp=hp, s=s: (
                k[0] == "vt" and k[1] == hp and k[2] <= min(s, 2)))
            fin = emit_attention(hp, s, fin)
        pthunks = make_proj(s)
        if s == 2:
            held.extend(pthunks)
        else:
            filler.extend((("proj", s), th) for th in pthunks)
    fin()

    while filler:
        pop_filler()
    for th in held:
        th()


def build():
    if "nc" in _CACHE:
        return _CACHE["nc"]
    nc = bacc.Bacc("TRN2", target_bir_lowering=False, debug=False,
                   num_devices=NCORES)
    with tile.TileContext(nc) as tc:
        _emit(tc)
    nc.compile()
    _CACHE["nc"] = nc
    return nc


def make_in_maps(x, qkv_w, qkv_b, proj_w):
    import ml_dtypes
    bf16 = ml_dtypes.bfloat16
    fp8 = ml_dtypes.float8_e4m3
    x = np.asarray(x, dtype=np.float32)
    qkv_w = np.asarray(qkv_w, dtype=np.float32)
    qkv_b = np.asarray(qkv_b, dtype=np.float32)
    proj_w = np.asarray(proj_w, dtype=np.float32)

    xTs = [np.ascontiguousarray(x[b].T).astype(fp8) for b in range(B)]
    xTbs = [np.ascontiguousarray(x[b].T).astype(bf16) for b in range(B)]
    tri = (np.arange(128)[None, :] >= np.arange(128)[:, None]).astype(bf16)
    tri2 = np.ascontiguousarray(np.tile(tri, (1, 2)))

    in_maps = []
    for c in range(NCORES):
        b, hg = c // 4, c % 4
        s = 256 * hg  # first feature row of this core's 4 heads
        blocks_qk, blocks_v, blocks_b = [], [], []
        for hp in range(2):
            f = s + 128 * hp
            for m in range(3):
                blk = qkv_w[:, m * C + f:m * C + f + 128]
                if m < 2:
                    blocks_qk.append(blk)
                else:
                    blocks_v.append(blk)
                blocks_b.append(qkv_b[m * C + f:m * C + f + 128])
        # q/k weights x32 so fp8 e4m3 stays out of the subnormal range;
        # the on-device bias step rescales by 1/32. v stays bf16.
        wqkv_c = np.ascontiguousarray(
            np.concatenate(blocks_qk, axis=1) * 32.0).astype(fp8)
        wv_c = np.ascontiguousarray(
            np.concatenate(blocks_v, axis=1)).astype(bf16)
        bqkv_c = np.ascontiguousarray(np.stack(blocks_b, axis=1))
        wp_c = np.ascontiguousarray(
            proj_w[s:s + 256, :].reshape(2, 128, C).transpose(1, 0, 2)
        ).astype(bf16)
        in_maps.append({
            "xT": xTs[b], "xTb": xTbs[b], "wqkv": wqkv_c, "wv": wv_c,
            "bqkv": bqkv_c, "wp": wp_c, "tri2": tri2,
        })
    return in_maps


def kernel(x, qkv_w, qkv_b, proj_w, proj_b, _trace=False):
    nc = build()
    in_maps = make_in_maps(x, qkv_w, qkv_b, proj_w)
    res = run_bass_kernel_spmd(nc, in_maps, core_ids=list(range(NCORES)),
                               trace=_trace)
    acc = np.zeros((B, T, C), dtype=np.float64)
    for c in range(NCORES):
        acc[c // 4] += res.results[c]["outp"].astype(np.float64)
    acc += np.asarray(proj_b, dtype=np.float64)
    out = acc.astype(np.float32)
    _CACHE["last_results"] = res
    return out
